# revision 10
# baseline (speedup 1.0000x reference)
"""Trainium2 Bass kernel for nn_Keyattention (sparse_attention).

Reference computation (per batch b):
  key   = Wk @ x + bk                      [128, N]   (x = assp[b] as [C=256, N=16384])
  q     = blockdiag(Wq) @ protos[:,b,:].T  [30, 128]  (class g = p // 5)
  sim   = q @ key                          [30, N]
  w     = max_p softmax_p(sim / 6)         [N]
  out   = x * w                            [C, N]
Outputs: (out, q, sim) as full tensors.

Sharding: data-parallel over batch B=8 across 8 cores; weights replicated.

Per-core dataflow (per 2048-pixel chunk):
  PE:  key GEMM -> sim (natural [30,n]) -> simT (pixel-partition [128,30] blocks)
       -> w broadcast (ones \otimes w-row)
  ACT: key bias-add copy, sim copy, exp(sim/6)
  DVE: rowmax/rowsum/recip -> w = max/sum; final x*w multiply
  max_p softmax_p == exp(max)/sum(exp) -- no max-subtraction needed (|sim/6| < ~9).
"""

import sys

import numpy as np

sys.path.insert(0, "/opt/trn_rl_repo")

from contextlib import ExitStack

import concourse.bass as bass
import concourse.bacc as bacc_mod
from concourse import mybir
from concourse.tile import TileContext

B, C, H, W = 8, 256, 128, 128
N = H * W                 # 16384 pixels per batch
P = 30                    # prototypes
NCLS = 6                  # classes
G = P // NCLS             # 5 prototypes per class
K = 128                   # key/query feature dim
SCALE = G / P             # 1/6 softmax temperature

CHUNK = 2048              # pixels per pipeline chunk
SUB = 512                 # matmul free-dim (one fp32 PSUM bank)
NSUB = CHUNK // SUB
NCHUNK = N // CHUNK
NGRP = CHUNK // 128       # 128-pixel groups per chunk (transposed softmax)

F32 = mybir.dt.float32
F32R = mybir.dt.float32r

_CACHED = {}


def _build_bass(use_f32r=True):
    """Build the per-core Bass module (SPMD across 8 cores)."""
    nc = bacc_mod.Bacc("TRN2", target_bir_lowering=False)

    XDT = F32R if use_f32r else F32

    x_d = nc.dram_tensor("x", [C, N], XDT, kind="ExternalInput")
    wk_d = nc.dram_tensor("wkT", [128, 2, K], XDT, kind="ExternalInput")
    wq_d = nc.dram_tensor("wqT", [128, 2, NCLS, K], F32, kind="ExternalInput")
    proto_d = nc.dram_tensor("protoT", [128, 2, P], F32, kind="ExternalInput")
    bk_d = nc.dram_tensor("bk", [K, 1], F32, kind="ExternalInput")
    bq_d = nc.dram_tensor("bqT", [K, P], F32, kind="ExternalInput")
    ident_d = nc.dram_tensor("ident", [128, 128], F32, kind="ExternalInput")
    identr_d = nc.dram_tensor("identr", [128, 128], F32R, kind="ExternalInput")
    ones_d = nc.dram_tensor("ones", [1, 128], F32R, kind="ExternalInput")

    out_d = nc.dram_tensor("out_w", [C, N], F32, kind="ExternalOutput")
    q_d = nc.dram_tensor("out_q", [P, K], F32, kind="ExternalOutput")
    sim_d = nc.dram_tensor("out_sim", [P, N], F32, kind="ExternalOutput")

    with TileContext(nc) as tc, ExitStack() as ctx:
        singles = ctx.enter_context(tc.tile_pool(name="singles", bufs=1))
        inpool = ctx.enter_context(tc.tile_pool(name="inpool", bufs=3))
        keypool = ctx.enter_context(tc.tile_pool(name="keypool", bufs=2))
        simpool = ctx.enter_context(tc.tile_pool(name="simpool", bufs=3))
        outpool = ctx.enter_context(tc.tile_pool(name="outpool", bufs=3))
        statpool = ctx.enter_context(tc.tile_pool(name="statpool", bufs=4))

        ps_key = ctx.enter_context(tc.tile_pool(name="ps_key", bufs=2, space="PSUM"))
        ps_sim = ctx.enter_context(tc.tile_pool(name="ps_sim", bufs=2, space="PSUM"))
        ps_simT = ctx.enter_context(tc.tile_pool(name="ps_simT", bufs=1, space="PSUM"))
        ps_small = ctx.enter_context(tc.tile_pool(name="ps_small", bufs=1, space="PSUM"))
        ps_wb = ctx.enter_context(tc.tile_pool(name="ps_wb", bufs=2, space="PSUM"))

        # ---- replicated weights / constants ----
        wk_sb = singles.tile([128, 2, K], XDT)
        nc.sync.dma_start(out=wk_sb, in_=wk_d[:])
        wq_sb = singles.tile([128, 2, NCLS, K], F32)
        nc.sync.dma_start(out=wq_sb, in_=wq_d[:])
        proto_sb = singles.tile([128, 2, P], F32)
        nc.sync.dma_start(out=proto_sb, in_=proto_d[:])
        bk_sb = singles.tile([K, 1], F32)
        nc.sync.dma_start(out=bk_sb, in_=bk_d[:])
        bq_sb = singles.tile([K, P], F32)
        nc.sync.dma_start(out=bq_sb, in_=bq_d[:])
        ident_sb = singles.tile([128, 128], F32)
        nc.sync.dma_start(out=ident_sb, in_=ident_d[:])
        identr_sb = singles.tile([128, 128], F32R)
        nc.sync.dma_start(out=identr_sb, in_=identr_d[:])
        ones_sb = singles.tile([1, 128], F32R)
        nc.sync.dma_start(out=ones_sb, in_=ones_d[:])
        zbias_sb = singles.tile([128, 1], F32)
        nc.vector.memset(zbias_sb, 0.0)

        # ---- q projection: qT[k, p] (block-diagonal over classes) ----
        qT_ps = ps_small.tile([K, 128], F32, tag="small")
        for g in range(NCLS):
            for ct in range(2):
                nc.tensor.matmul(
                    qT_ps[:, g * G:(g + 1) * G],
                    lhsT=wq_sb[:, ct, g, :],
                    rhs=proto_sb[:, ct, g * G:(g + 1) * G],
                    start=(ct == 0),
                    stop=(ct == 1),
                )
        qT_sb = singles.tile([K, P], F32)
        nc.vector.tensor_add(qT_sb, qT_ps[:, :P], bq_sb)
        qT_r = singles.tile([K, P], XDT)
        nc.vector.tensor_copy(qT_r, qT_sb)

        q_ps = ps_small.tile([P, 128], F32, tag="small")
        nc.tensor.transpose(q_ps, qT_sb, ident_sb)
        q_sb = singles.tile([P, K], F32)
        nc.scalar.copy(q_sb, q_ps[:, :K])
        nc.sync.dma_start(out=q_d[:], in_=q_sb)

        # ---- main streaming loop over pixel chunks ----
        for ci in range(NCHUNK):
            n0 = ci * CHUNK

            x0 = inpool.tile([128, CHUNK], XDT, tag="x0")
            x1 = inpool.tile([128, CHUNK], XDT, tag="x1")
            nc.sync.dma_start(out=x0, in_=x_d[0:128, n0:n0 + CHUNK])
            nc.sync.dma_start(out=x1, in_=x_d[128:256, n0:n0 + CHUNK])

            # key = WkT.T @ x + bk  -> SBUF [128, CHUNK]
            key_sb = keypool.tile([K, CHUNK], XDT, tag="key")
            for s in range(NSUB):
                sl = slice(s * SUB, (s + 1) * SUB)
                kps = ps_key.tile([K, SUB], F32, tag="kps")
                nc.tensor.matmul(kps, lhsT=wk_sb[:, 0, :], rhs=x0[:, sl],
                                 start=True, stop=False)
                nc.tensor.matmul(kps, lhsT=wk_sb[:, 1, :], rhs=x1[:, sl],
                                 start=False, stop=True)
                nc.scalar.activation(key_sb[:, sl], kps,
                                     mybir.ActivationFunctionType.Identity,
                                     bias=bk_sb, scale=1.0)

            # sim (natural [30, CHUNK]): DMA to DRAM straight from PSUM;
            # exp_nat = exp(sim/6) as f32r (rounding exp, not sim) for the
            # transposed softmax path
            exp_sb = simpool.tile([P, CHUNK], F32R, tag="exp")
            sim_sb = simpool.tile([P, CHUNK], F32, tag="sim")
            for s in range(NSUB):
                sl = slice(s * SUB, (s + 1) * SUB)
                sps = ps_sim.tile([P, SUB], F32, tag="sps")
                nc.tensor.matmul(sps, lhsT=qT_r, rhs=key_sb[:, sl],
                                 start=True, stop=True)
                nc.scalar.copy(sim_sb[:, sl], sps)
                nc.scalar.activation(exp_sb[:, sl], sps,
                                     mybir.ActivationFunctionType.Exp,
                                     bias=zbias_sb[:P], scale=float(SCALE))
            nc.sync.dma_start(out=sim_d[:, n0:n0 + CHUNK], in_=sim_sb)

            # expT [128, NGRP*30] (pixels -> partitions) via f32r PE transpose
            tps = ps_simT.tile([128, NGRP * P], F32R, tag="tps")
            for gi in range(NGRP):
                nc.tensor.transpose(tps[:, gi * P:(gi + 1) * P],
                                    exp_sb[:, gi * 128:(gi + 1) * 128],
                                    identr_sb[:P, :P])

            # w = rowmax(E) / rowsum(E), E read straight from PSUM
            e_v = tps.bitcast(F32).rearrange("a (b c) -> a b c", c=P)
            m_sb = statpool.tile([128, NGRP], F32, tag="m")
            s_sb = statpool.tile([128, NGRP], F32, tag="s")
            r_sb = statpool.tile([128, NGRP], F32, tag="r")
            w_sb = statpool.tile([128, NGRP], F32, tag="w")
            nc.vector.reduce_max(m_sb, e_v, axis=mybir.AxisListType.X)
            nc.vector.reduce_sum(s_sb, e_v, axis=mybir.AxisListType.X)
            nc.vector.reciprocal(r_sb, s_sb)
            nc.vector.tensor_mul(w_sb, m_sb, r_sb)

            # wT rows: w for 128 consecutive pixels per row; gather to one row
            wT_ps = ps_small.tile([NGRP, 128], F32, tag="small")
            nc.tensor.transpose(wT_ps, w_sb, ident_sb)
            wT_sb = statpool.tile([NGRP, 128], F32R, tag="wts")
            nc.scalar.copy(wT_sb, wT_ps)
            w_row = statpool.tile([1, CHUNK], F32R, tag="wrow")
            nc.sync.dma_start(
                out=w_row.rearrange("a (b c) -> a b c", b=NGRP, c=128),
                in_=wT_sb)

            # broadcast w across partitions (ones \otimes w-row), multiply
            out0 = outpool.tile([128, CHUNK], F32, tag="o0")
            out1 = outpool.tile([128, CHUNK], F32, tag="o1")
            for s in range(NSUB):
                sl = slice(s * SUB, (s + 1) * SUB)
                wb = ps_wb.tile([128, SUB], F32, tag="wb")
                nc.tensor.matmul(wb, lhsT=ones_sb, rhs=w_row[:, sl],
                                 start=True, stop=True)
                nc.vector.tensor_mul(out0[:, sl], x0[:, sl].bitcast(F32), wb)
                nc.vector.tensor_mul(out1[:, sl], x1[:, sl].bitcast(F32), wb)
            nc.sync.dma_start(out=out_d[0:128, n0:n0 + CHUNK], in_=out0)
            nc.sync.dma_start(out=out_d[128:256, n0:n0 + CHUNK], in_=out1)

    nc.compile()
    return nc


def _host_prep(assp, prototypes, Wk, bk, Wq, bq):
    """Build per-core input maps (numpy, fp32)."""
    WkT = np.ascontiguousarray(Wk.T)                        # [C, K]
    wkT = np.ascontiguousarray(
        WkT.reshape(2, 128, K).transpose(1, 0, 2))          # [128, 2, K]
    WqT = np.ascontiguousarray(Wq.transpose(2, 0, 1))       # [C, NCLS, K]
    wqT = np.ascontiguousarray(
        WqT.reshape(2, 128, NCLS, K).transpose(1, 0, 2, 3))  # [128,2,NCLS,K]
    bkv = np.ascontiguousarray(bk.reshape(K, 1))
    bqT = np.ascontiguousarray(np.repeat(bq, G, axis=0).T)  # [K, P]
    ident = np.eye(128, dtype=np.float32)

    in_maps = []
    for b in range(B):
        x = np.ascontiguousarray(assp[b].reshape(C, N))
        protoT = prototypes[:, b, :].T                      # [C, P]
        protoT = np.ascontiguousarray(
            protoT.reshape(2, 128, P).transpose(1, 0, 2))   # [128, 2, P]
        in_maps.append({
            "x": x.astype(np.float32),
            "wkT": wkT.astype(np.float32),
            "wqT": wqT.astype(np.float32),
            "protoT": protoT.astype(np.float32),
            "bk": bkv.astype(np.float32),
            "bqT": bqT.astype(np.float32),
            "ident": ident,
            "identr": ident,
            "ones": np.ones((1, 128), np.float32),
        })
    return in_maps


def _run(inputs, trace=False, use_f32r=True):
    from concourse.bass_utils import run_bass_kernel_spmd

    key = ("nc", use_f32r)
    if key not in _CACHED:
        _CACHED[key] = _build_bass(use_f32r)
    nc = _CACHED[key]

    in_maps = _host_prep(
        np.asarray(inputs["assp_features"], np.float32),
        np.asarray(inputs["prototypes"], np.float32),
        np.asarray(inputs["Wk"], np.float32),
        np.asarray(inputs["bk"], np.float32),
        np.asarray(inputs["Wq"], np.float32),
        np.asarray(inputs["bq"], np.float32),
    )
    res = run_bass_kernel_spmd(nc, in_maps, core_ids=list(range(B)), trace=trace)

    outs = res.results
    assp_w = np.stack([outs[b]["out_w"].reshape(C, H, W) for b in range(B)])
    q = np.stack([outs[b]["out_q"] for b in range(B)], axis=1)   # [P, B, K]
    sim = np.stack([outs[b]["out_sim"].reshape(P, H, W) for b in range(B)])
    return (assp_w, q, sim), res


def kernel(**inputs):
    (assp_w, q, sim), _ = _run(inputs, trace=False)
    return assp_w, q, sim


# revision 12
# speedup vs baseline: 1.1962x; 1.1962x over previous
"""Trainium2 Bass kernel for nn_Keyattention (sparse_attention).

Reference computation (per batch b):
  key   = Wk @ x + bk                      [128, N]   (x = assp[b] as [C=256, N=16384])
  q     = blockdiag(Wq) @ protos[:,b,:].T  [30, 128]  (class g = p // 5)
  sim   = q @ key                          [30, N]
  w     = max_p softmax_p(sim / 6)         [N]
  out   = x * w                            [C, N]
Outputs: (out, q, sim) as full tensors.

Sharding: data-parallel over batch B=8 across 8 cores; weights replicated.

Per-core dataflow (per 2048-pixel chunk):
  PE:  key GEMM -> sim (natural [30,n]) -> simT (pixel-partition [128,30] blocks)
       -> w broadcast (ones \otimes w-row)
  ACT: key bias-add copy, sim copy, exp(sim/6)
  DVE: rowmax/rowsum/recip -> w = max/sum; final x*w multiply
  max_p softmax_p == exp(max)/sum(exp) -- no max-subtraction needed (|sim/6| < ~9).
"""

import sys

import numpy as np

sys.path.insert(0, "/opt/trn_rl_repo")

from contextlib import ExitStack

import concourse.bass as bass
import concourse.bacc as bacc_mod
from concourse import mybir
from concourse.tile import TileContext

B, C, H, W = 8, 256, 128, 128
N = H * W                 # 16384 pixels per batch
P = 30                    # prototypes
NCLS = 6                  # classes
G = P // NCLS             # 5 prototypes per class
K = 128                   # key/query feature dim
SCALE = G / P             # 1/6 softmax temperature

CHUNK = 2048              # pixels per pipeline chunk
SUB = 512                 # matmul free-dim (one fp32 PSUM bank)
NSUB = CHUNK // SUB
NCHUNK = N // CHUNK
NGRP = CHUNK // 128       # 128-pixel groups per chunk (transposed softmax)

F32 = mybir.dt.float32
F32R = mybir.dt.float32r

_CACHED = {}


def _build_bass(use_f32r=True):
    """Build the per-core Bass module (SPMD across 8 cores)."""
    nc = bacc_mod.Bacc("TRN2", target_bir_lowering=False)

    XDT = F32R if use_f32r else F32

    x_d = nc.dram_tensor("x", [C, N], XDT, kind="ExternalInput")
    wk_d = nc.dram_tensor("wkT", [128, 2, K], XDT, kind="ExternalInput")
    wq_d = nc.dram_tensor("wqT", [128, 2, NCLS, K], F32, kind="ExternalInput")
    proto_d = nc.dram_tensor("protoT", [128, 2, P], F32, kind="ExternalInput")
    bk_d = nc.dram_tensor("bk", [K, 1], F32, kind="ExternalInput")
    bq_d = nc.dram_tensor("bqT", [K, P], F32, kind="ExternalInput")
    ident_d = nc.dram_tensor("ident", [128, 128], F32, kind="ExternalInput")
    identr_d = nc.dram_tensor("identr", [128, 128], F32R, kind="ExternalInput")
    ones_d = nc.dram_tensor("ones", [1, 128], F32R, kind="ExternalInput")

    out_d = nc.dram_tensor("out_w", [C, N], F32, kind="ExternalOutput")
    q_d = nc.dram_tensor("out_q", [P, K], F32, kind="ExternalOutput")
    sim_d = nc.dram_tensor("out_sim", [P, N], F32, kind="ExternalOutput")

    with TileContext(nc) as tc, ExitStack() as ctx:
        singles = ctx.enter_context(tc.tile_pool(name="singles", bufs=1))
        inpool = ctx.enter_context(tc.tile_pool(name="inpool", bufs=3))
        keypool = ctx.enter_context(tc.tile_pool(name="keypool", bufs=2))
        simpool = ctx.enter_context(tc.tile_pool(name="simpool", bufs=3))
        outpool = ctx.enter_context(tc.tile_pool(name="outpool", bufs=3))
        statpool = ctx.enter_context(tc.tile_pool(name="statpool", bufs=4))

        ps_key = ctx.enter_context(tc.tile_pool(name="ps_key", bufs=2, space="PSUM"))
        ps_sim = ctx.enter_context(tc.tile_pool(name="ps_sim", bufs=2, space="PSUM"))
        ps_simT = ctx.enter_context(tc.tile_pool(name="ps_simT", bufs=1, space="PSUM"))
        ps_small = ctx.enter_context(tc.tile_pool(name="ps_small", bufs=1, space="PSUM"))
        ps_wb = ctx.enter_context(tc.tile_pool(name="ps_wb", bufs=2, space="PSUM"))

        # ---- replicated weights / constants ----
        wk_sb = singles.tile([128, 2, K], XDT)
        nc.sync.dma_start(out=wk_sb, in_=wk_d[:])
        wq_sb = singles.tile([128, 2, NCLS, K], F32)
        nc.sync.dma_start(out=wq_sb, in_=wq_d[:])
        proto_sb = singles.tile([128, 2, P], F32)
        nc.sync.dma_start(out=proto_sb, in_=proto_d[:])
        bk_sb = singles.tile([K, 1], F32)
        nc.sync.dma_start(out=bk_sb, in_=bk_d[:])
        bq_sb = singles.tile([K, P], F32)
        nc.sync.dma_start(out=bq_sb, in_=bq_d[:])
        ident_sb = singles.tile([128, 128], F32)
        nc.sync.dma_start(out=ident_sb, in_=ident_d[:])
        identr_sb = singles.tile([128, 128], F32R)
        nc.sync.dma_start(out=identr_sb, in_=identr_d[:])
        ones_sb = singles.tile([1, 128], F32R)
        nc.sync.dma_start(out=ones_sb, in_=ones_d[:])
        zbias_sb = singles.tile([128, 1], F32)
        nc.vector.memset(zbias_sb, 0.0)

        # ---- main streaming loop over pixel chunks ----
        def load_x(ci):
            n0 = ci * CHUNK
            x0 = inpool.tile([128, CHUNK], XDT, tag="x0")
            x1 = inpool.tile([128, CHUNK], XDT, tag="x1")
            nc.sync.dma_start(out=x0, in_=x_d[0:128, n0:n0 + CHUNK])
            nc.sync.dma_start(out=x1, in_=x_d[128:256, n0:n0 + CHUNK])
            return x0, x1

        xs = load_x(0)
        # ---- q projection: qT[k, p] (block-diagonal over classes) ----
        qT_ps = ps_small.tile([K, 128], F32, tag="small")
        for g in range(NCLS):
            for ct in range(2):
                nc.tensor.matmul(
                    qT_ps[:, g * G:(g + 1) * G],
                    lhsT=wq_sb[:, ct, g, :],
                    rhs=proto_sb[:, ct, g * G:(g + 1) * G],
                    start=(ct == 0),
                    stop=(ct == 1),
                )
        qT_sb = singles.tile([K, P], F32)
        nc.vector.tensor_add(qT_sb, qT_ps[:, :P], bq_sb)
        qT_r = singles.tile([K, P], XDT)
        nc.vector.tensor_copy(qT_r, qT_sb)

        q_ps = ps_small.tile([P, 128], F32, tag="small")
        nc.tensor.transpose(q_ps, qT_sb, ident_sb)
        q_sb = singles.tile([P, K], F32)
        nc.scalar.copy(q_sb, q_ps[:, :K])
        nc.sync.dma_start(out=q_d[:], in_=q_sb)

        for ci in range(NCHUNK):
            n0 = ci * CHUNK
            x0, x1 = xs
            if ci + 1 < NCHUNK:
                xs = load_x(ci + 1)

            # key = WkT.T @ x + bk  -> SBUF [128, CHUNK]
            key_sb = keypool.tile([K, CHUNK], XDT, tag="key")
            for s in range(NSUB):
                sl = slice(s * SUB, (s + 1) * SUB)
                kps = ps_key.tile([K, SUB], F32, tag="kps")
                nc.tensor.matmul(kps, lhsT=wk_sb[:, 0, :], rhs=x0[:, sl],
                                 start=True, stop=False)
                nc.tensor.matmul(kps, lhsT=wk_sb[:, 1, :], rhs=x1[:, sl],
                                 start=False, stop=True)
                nc.scalar.activation(key_sb[:, sl], kps,
                                     mybir.ActivationFunctionType.Identity,
                                     bias=bk_sb, scale=1.0)

            # sim (natural [30, CHUNK]): DMA to DRAM straight from PSUM;
            # exp_nat = exp(sim/6) as f32r (rounding exp, not sim) for the
            # transposed softmax path
            exp_sb = simpool.tile([P, CHUNK], F32R, tag="exp")
            sim_sb = simpool.tile([P, CHUNK], F32, tag="sim")
            for s in range(NSUB):
                sl = slice(s * SUB, (s + 1) * SUB)
                sps = ps_sim.tile([P, SUB], F32, tag="sps")
                nc.tensor.matmul(sps, lhsT=qT_r, rhs=key_sb[:, sl],
                                 start=True, stop=True)
                nc.scalar.copy(sim_sb[:, sl], sps)
                nc.scalar.activation(exp_sb[:, sl], sps,
                                     mybir.ActivationFunctionType.Exp,
                                     bias=zbias_sb[:P], scale=float(SCALE))
            nc.sync.dma_start(out=sim_d[:, n0:n0 + CHUNK], in_=sim_sb)

            # expT [128, NGRP*30] (pixels -> partitions) via f32r PE transpose
            tps = ps_simT.tile([128, NGRP * P], F32R, tag="tps")
            for gi in range(NGRP):
                nc.tensor.transpose(tps[:, gi * P:(gi + 1) * P],
                                    exp_sb[:, gi * 128:(gi + 1) * 128],
                                    identr_sb[:P, :P])

            # w = rowmax(E) / rowsum(E), E read straight from PSUM
            e_v = tps.bitcast(F32).rearrange("a (b c) -> a b c", c=P)
            m_sb = statpool.tile([128, NGRP], F32, tag="m")
            s_sb = statpool.tile([128, NGRP], F32, tag="s")
            r_sb = statpool.tile([128, NGRP], F32, tag="r")
            w_sb = statpool.tile([128, NGRP], F32, tag="w")
            nc.vector.reduce_max(m_sb, e_v, axis=mybir.AxisListType.X)
            nc.vector.reduce_sum(s_sb, e_v, axis=mybir.AxisListType.X)
            nc.vector.reciprocal(r_sb, s_sb)
            nc.vector.tensor_mul(w_sb, m_sb, r_sb)

            # wT rows: w for 128 consecutive pixels per row; gather to one row
            wT_ps = ps_small.tile([NGRP, 128], F32, tag="small")
            nc.tensor.transpose(wT_ps, w_sb, ident_sb)
            wT_sb = statpool.tile([NGRP, 128], F32R, tag="wts")
            nc.scalar.copy(wT_sb, wT_ps)
            w_row = statpool.tile([1, CHUNK], F32R, tag="wrow")
            nc.sync.dma_start(
                out=w_row.rearrange("a (b c) -> a b c", b=NGRP, c=128),
                in_=wT_sb)

            # broadcast w across partitions (ones \otimes w-row), multiply
            out0 = outpool.tile([128, CHUNK], F32, tag="o0")
            out1 = outpool.tile([128, CHUNK], F32, tag="o1")
            for s in range(NSUB):
                sl = slice(s * SUB, (s + 1) * SUB)
                wb = ps_wb.tile([128, SUB], F32, tag="wb")
                nc.tensor.matmul(wb, lhsT=ones_sb, rhs=w_row[:, sl],
                                 start=True, stop=True)
                nc.vector.tensor_mul(out0[:, sl], x0[:, sl].bitcast(F32), wb)
                nc.vector.tensor_mul(out1[:, sl], x1[:, sl].bitcast(F32), wb)
            nc.sync.dma_start(out=out_d[0:128, n0:n0 + CHUNK], in_=out0)
            nc.sync.dma_start(out=out_d[128:256, n0:n0 + CHUNK], in_=out1)

    nc.compile()
    return nc


def _host_prep(assp, prototypes, Wk, bk, Wq, bq):
    """Build per-core input maps (numpy, fp32)."""
    WkT = np.ascontiguousarray(Wk.T)                        # [C, K]
    wkT = np.ascontiguousarray(
        WkT.reshape(2, 128, K).transpose(1, 0, 2))          # [128, 2, K]
    WqT = np.ascontiguousarray(Wq.transpose(2, 0, 1))       # [C, NCLS, K]
    wqT = np.ascontiguousarray(
        WqT.reshape(2, 128, NCLS, K).transpose(1, 0, 2, 3))  # [128,2,NCLS,K]
    bkv = np.ascontiguousarray(bk.reshape(K, 1))
    bqT = np.ascontiguousarray(np.repeat(bq, G, axis=0).T)  # [K, P]
    ident = np.eye(128, dtype=np.float32)

    in_maps = []
    for b in range(B):
        x = np.ascontiguousarray(assp[b].reshape(C, N))
        protoT = prototypes[:, b, :].T                      # [C, P]
        protoT = np.ascontiguousarray(
            protoT.reshape(2, 128, P).transpose(1, 0, 2))   # [128, 2, P]
        in_maps.append({
            "x": x.astype(np.float32),
            "wkT": wkT.astype(np.float32),
            "wqT": wqT.astype(np.float32),
            "protoT": protoT.astype(np.float32),
            "bk": bkv.astype(np.float32),
            "bqT": bqT.astype(np.float32),
            "ident": ident,
            "identr": ident,
            "ones": np.ones((1, 128), np.float32),
        })
    return in_maps


def _run(inputs, trace=False, use_f32r=True):
    from concourse.bass_utils import run_bass_kernel_spmd

    key = ("nc", use_f32r)
    if key not in _CACHED:
        _CACHED[key] = _build_bass(use_f32r)
    nc = _CACHED[key]

    in_maps = _host_prep(
        np.asarray(inputs["assp_features"], np.float32),
        np.asarray(inputs["prototypes"], np.float32),
        np.asarray(inputs["Wk"], np.float32),
        np.asarray(inputs["bk"], np.float32),
        np.asarray(inputs["Wq"], np.float32),
        np.asarray(inputs["bq"], np.float32),
    )
    res = run_bass_kernel_spmd(nc, in_maps, core_ids=list(range(B)), trace=trace)

    outs = res.results
    assp_w = np.stack([outs[b]["out_w"].reshape(C, H, W) for b in range(B)])
    q = np.stack([outs[b]["out_q"] for b in range(B)], axis=1)   # [P, B, K]
    sim = np.stack([outs[b]["out_sim"].reshape(P, H, W) for b in range(B)])
    return (assp_w, q, sim), res


def kernel(**inputs):
    (assp_w, q, sim), _ = _run(inputs, trace=False)
    return assp_w, q, sim
